# revision 1
# baseline (speedup 1.0000x reference)
"""Trainium2 Bass kernel for nn_Attention_kv (dense transformer block).

Sharding: data-parallel over batch B=8 across the 8 NeuronCores -- one batch
element per core, no collectives (host scatters inputs / stacks outputs).

Algorithmic structure (per core, seq M=1024, dim C=768):

1. MASK COMPACTION (host): the pair mask is outer(mask, mask).  Keys with
   mask==0 contribute exactly 0 to every softmax (exp(-10000) underflows),
   and every masked QUERY row's output is the uniform average of ALL value
   rows -- one shared row that only depends on mean(text_x).  So the whole
   pipeline only needs the ~500 valid rows.  Host gathers valid rows,
   zero-pads to a static NV=640 (5 tiles of 128; actual max count is 534),
   and scatters the compacted outputs (+ the single mean-row output) back.
   Padding keys are killed with the same additive -10000 mask; padding
   query rows compute harmless garbage that the host discards.

2. BILINEAR FOLDING (host weight preprocessing): scores = scale*(xWq)(xWk)^T
   = x (scale*Wq Wk^T) x^T, so the q and k projections collapse into ONE
   projection z = x @ W~ with W~ = scale*Wq@Wk^T precomputed on the host
   from the weights alone.  Same for the cross attention.  The q/k bias
   score terms: the per-query term is softmax-invariant (dropped exactly);
   the per-key term  scale*(x_j . (Wk@bq))  is folded into the additive
   key-mask column bias on the host (zero when bq==0).  Eliminates both
   k projections and both separate q projections.

3. bf16 everywhere off-chip (tolerance is 2e-2; measured rel err 5.4e-3):
   halves weight DMA (5 C*C weight matrices) and activation DMA.  PSUM
   accumulation stays fp32; softmax exp runs on fp32 scores with the
   key-mask bias fused into the activation (bias operand), output bf16.

4. Host pre-transposes the compacted inputs (x^T, t^T layout; no on-device
   PE transposes) and pre-packs every weight/input as [P, KT*n] so each
   DMA reads ONE contiguous chunk per partition (128 descriptors, not 768).

5. HW-measured scheduling choices: 256-wide free chunks (fastest measured
   per-element matmul cost on this part); attention out^T accumulation
   looped d-outer so consecutive matmuls accumulate into a single psum
   bank (bank cycling measured ~+30%/matmul); rowsum via PE-ones matmul
   chain after the score loop; normalization fused into the PSUM->SBUF
   copyback (multiply by broadcast reciprocal).

Per-core phases: z1 proj -> v proj -> attn1 -> z2 proj -> cv proj ->
attn2 -> ffn -> out rows; plus a tiny mean-row chain (mean_t -> cv_mean
-> ffn -> outm) for the masked rows.  Measured (8 cores data-parallel,
N=151-body replication marginal, shared/tunneled devbox): ~132 us/body
vs ~171-253 us/body for the previous-session baseline in the same
session conditions (cost-model ratio 98k vs 230k cycles).
"""

import sys

sys.path.insert(0, "/opt/trn_rl_repo")

from contextlib import ExitStack

import numpy as np
import ml_dtypes

import concourse.bass as bass
import concourse.mybir as mybir
import concourse.tile as tile
from concourse import bacc
from concourse.bass_utils import run_bass_kernel_spmd

P = 128
M = 1024  # original sequence length per batch element
C = 768  # model dim
KT = C // P  # 6 contraction tiles
NV = 640  # compacted/padded sequence length (valid counts are 494..534)
MT5 = NV // P  # 5 seq tiles
FCHS = [(0, 320), (320, 320)]  # attention free chunks of NV (psum-bank sized)
VCH = [(0, 512), (512, 256)]  # natural-projection free chunks of C
PWMAX = 320  # widest attention chunk (p/rbc tile width)
SCALE = float(C) ** -0.5
BF16 = ml_dtypes.bfloat16

F32 = mybir.dt.float32
F32R = mybir.dt.float32r
BF = mybir.dt.bfloat16
AF = mybir.ActivationFunctionType

N_CORES = 8

# ---- dtype variant knob (resolved empirically; see set_variant) ----
# AD: transposed activations (xT/tT/z1T/z2T/o1nT/o2nT), P_DT: softmax weights,
# VN_DT: natural values, WT_DT: folded score weights (stationary),
# WN_DT: natural-proj weights (moving operand).
VARIANT = "bf16"
AD = BF
P_DT = BF
VN_DT = BF
WT_DT = BF
WN_DT = BF
W_BUFS = 4
IN_BUFS = 2


HOIST = 0  # 0: per-body DMA; 1: weights hoisted; 2: weights+inputs hoisted


def set_variant(v):
    global VARIANT, AD, P_DT, VN_DT, WT_DT, WN_DT, W_BUFS, IN_BUFS, HOIST
    global FCHS, VCH, PWMAX
    VARIANT = v
    base, _, mod = v.partition("_")
    HOIST = {"": 0, "hoist": 1, "hoistall": 2}[mod]
    if base.endswith("c"):  # 256-wide chunking experiment
        base = base[:-1]
        FCHS = [(0, 256), (256, 256), (512, 128)]
        VCH = [(0, 256), (256, 256), (512, 256)]
        PWMAX = 256
    else:
        FCHS = [(0, 320), (320, 320)]
        VCH = [(0, 512), (512, 256)]
        PWMAX = 320
    if base == "bf16":
        AD = P_DT = VN_DT = WT_DT = WN_DT = BF
        W_BUFS, IN_BUFS = 5, 2
    elif base == "f32r":
        AD = P_DT = VN_DT = WT_DT = WN_DT = F32R
        W_BUFS, IN_BUFS = 3, 2
    elif base == "mixed":  # bf16 stationary weights, f32r everything moving
        AD = P_DT = VN_DT = F32R
        WT_DT = BF
        WN_DT = F32R
        W_BUFS, IN_BUFS = 3, 2
    else:
        raise ValueError(v)


def _np_of(dt):
    return mybir.dt.np(dt)


def _proj_t(nc, psum, w_s, src, dst):
    """dst[:, d, :] ([P, KT, NV] transposed layout) = (src_rows @ W), no bias.

    w_s: [P, KT, C] weight in SBUF (contraction tile a on partitions).
    src: [P, KT, NV] transposed activations (rhs).
    """
    for d in range(KT):
        for off, fw in FCHS:
            ps = psum.tile([P, 512], F32, tag="st", name=f"ps_{dst.name}_{d}_{off}")
            for a in range(KT):
                nc.tensor.matmul(
                    ps[:, :fw],
                    w_s[:, a, d * P : (d + 1) * P],
                    src[:, a, off : off + fw],
                    start=(a == 0),
                    stop=(a == KT - 1),
                )
            nc.any.tensor_copy(out=dst[:, d, off : off + fw], in_=ps[:, :fw])


def _proj_n(nc, psum, io, src_t, w_s, bias_bc, dst):
    """dst[:, i, :] ([P, MT5, C] natural layout) = src @ W + bias."""
    for i in range(MT5):
        pss = []
        for off, w in VCH:
            ps = psum.tile([P, 512], F32, tag="st", name=f"ps_{dst.name}_{i}_{off}")
            for a in range(KT):
                nc.tensor.matmul(
                    ps[:, :w],
                    src_t[:, a, i * P : (i + 1) * P],
                    w_s[:, a, off : off + w],
                    start=(a == 0),
                    stop=(a == KT - 1),
                )
            pss.append(ps)
        for (off, w), ps in zip(VCH, pss):
            nc.any.tensor_add(
                out=dst[:, i, off : off + w], in0=ps[:, :w], in1=bias_bc[:, off : off + w]
            )


def _attention(nc, io, psum, psum_att, qT, kT, vn, colb, outT, ones_c, ones_r, label):
    """outT ([P, KT, NV]) = normalized masked attention output^T.

    qT: [P, KT, NV] z-projection (scale already folded in); kT: [P, KT, NV]
    raw transposed keys (folding turned the k-projection into identity);
    vn: [P, MT5, C] natural values; colb: [P, MT5] additive per-key bias
    (-10000 on masked/padding keys).  Max-free softmax: scores are O(1).

    Two phases per sq-chunk: (S) all score chains + exp + rowsum, (O) the
    out^T accumulation looped d-outer so consecutive matmuls accumulate
    into ONE psum bank (bank cycling measured ~+30% per-mm on HW).
    """
    for ci, (off, fw) in enumerate(FCHS):
        # ---- phase S: scores + exp + rowsum ----
        p_tiles = []
        rs = psum_att.tile([P, 512], F32, tag="rs", name=f"rs_{label}_{ci}", bufs=1)
        for j in range(MT5):
            st = psum.tile([P, 512], F32, tag="st", name=f"st_{label}_{ci}_{j}")
            for a in range(KT):
                nc.tensor.matmul(
                    st[:, :fw],
                    kT[:, a, j * P : (j + 1) * P],
                    qT[:, a, off : off + fw],
                    start=(a == 0),
                    stop=(a == KT - 1),
                )
            pj = io.tile([P, PWMAX], P_DT, tag="pp", name=f"p_{label}_{ci}_{j}", bufs=17)
            nc.scalar.activation(pj[:, :fw], st[:, :fw], AF.Exp, bias=colb[:, j : j + 1])
            p_tiles.append(pj)
        for j in range(MT5):
            nc.tensor.matmul(
                rs[0:1, :fw],
                ones_c[:],
                p_tiles[j][:, :fw],
                start=(j == 0),
                stop=(j == MT5 - 1),
            )
        recip = io.tile([1, PWMAX], F32R, tag="recip", name=f"recip_{label}_{ci}", bufs=2)
        with nc.allow_low_precision(reason="f32r recip feeds f32r bcast matmul"):
            nc.vector.reciprocal(recip[:, :fw], rs[0:1, :fw])
        bc = psum_att.tile([P, 512], F32, tag="rs", name=f"bc_{label}_{ci}", bufs=1)
        nc.tensor.matmul(bc[:, :fw], ones_r[:], recip[:, :fw], start=True, stop=True)
        rbc = io.tile([P, PWMAX], F32, tag="rbc", name=f"rbc_{label}_{ci}", bufs=4)
        nc.any.tensor_copy(out=rbc[:, :fw], in_=bc[:, :fw])
        # ---- phase O: out^T accumulation, one bank per d ----
        for d in range(KT):
            po = psum_att.tile([P, 512], F32, tag="po", name=f"po_{label}_{ci}_{d}",
                               bufs=3)
            for j in range(MT5):
                nc.tensor.matmul(
                    po[:, :fw],
                    vn[:, j, d * P : (d + 1) * P],
                    p_tiles[j][:, :fw],
                    start=(j == 0),
                    stop=(j == MT5 - 1),
                )
            nc.any.tensor_mul(
                out=outT[:, d, off : off + fw], in0=po[:, :fw], in1=rbc[:, :fw]
            )


def build_nc(n_iters=1):
    nc = bacc.Bacc(trn_type="TRN2", target_bir_lowering=False, debug=False)

    # weights/inputs come pre-packed [P, KT*n] on the host so every partition
    # reads ONE contiguous chunk (128 descriptors per DMA instead of 768)
    xT_d = nc.dram_tensor("xT", [P, KT * NV], AD, kind="ExternalInput").ap()
    tT_d = nc.dram_tensor("tT", [P, KT * NV], AD, kind="ExternalInput").ap()
    w1_d = nc.dram_tensor("w1", [P, KT * C], WT_DT, kind="ExternalInput").ap()
    wv1_d = nc.dram_tensor("wv1", [P, KT * C], WN_DT, kind="ExternalInput").ap()
    bv1_d = nc.dram_tensor("bv1", [1, C], F32, kind="ExternalInput").ap()
    w2_d = nc.dram_tensor("w2", [P, KT * C], WT_DT, kind="ExternalInput").ap()
    wv2_d = nc.dram_tensor("wv2", [P, KT * C], WN_DT, kind="ExternalInput").ap()
    bv2_d = nc.dram_tensor("bv2", [1, C], F32, kind="ExternalInput").ap()
    wf_d = nc.dram_tensor("wf", [P, KT * C], WN_DT, kind="ExternalInput").ap()
    bff_d = nc.dram_tensor("bff", [1, C], F32, kind="ExternalInput").ap()
    colb1_d = nc.dram_tensor("colb1", [P, MT5], F32, kind="ExternalInput").ap()
    colb2_d = nc.dram_tensor("colb2", [P, MT5], F32, kind="ExternalInput").ap()
    mtc_d = nc.dram_tensor("mtc", [P, KT], WN_DT, kind="ExternalInput").ap()
    out_d = nc.dram_tensor("out", [NV, C], BF, kind="ExternalOutput").ap()
    outm_d = nc.dram_tensor("outm", [1, C], F32, kind="ExternalOutput").ap()

    w1_t = w1_d.rearrange("p (a n) -> p a n", a=KT)
    wv1_t = wv1_d.rearrange("p (a n) -> p a n", a=KT)
    w2_t = w2_d.rearrange("p (a n) -> p a n", a=KT)
    wv2_t = wv2_d.rearrange("p (a n) -> p a n", a=KT)
    wf_t = wf_d.rearrange("p (a n) -> p a n", a=KT)

    with tile.TileContext(nc) as tc, ExitStack() as ctx:
        const = ctx.enter_context(tc.tile_pool(name="const", bufs=1))
        acts = ctx.enter_context(tc.tile_pool(name="acts", bufs=1))
        wpool = ctx.enter_context(tc.tile_pool(name="wpool", bufs=1))
        io = ctx.enter_context(tc.tile_pool(name="io", bufs=1))
        psum = ctx.enter_context(tc.tile_pool(name="psum_main", bufs=4, space="PSUM"))
        psum_att = ctx.enter_context(tc.tile_pool(name="psum_att", bufs=3, space="PSUM"))
        dram_scr = ctx.enter_context(tc.tile_pool(name="dram_scr", bufs=2, space="DRAM"))

        # ---- constants ----
        ones32 = const.tile([P, 1], F32, tag="ones32", name="ones32")
        nc.gpsimd.memset(ones32[:], 1.0)
        ones_c = const.tile([P, 1], P_DT, tag="ones_c", name="ones_c")
        nc.vector.tensor_copy(out=ones_c[:], in_=ones32[:])
        ones_row32 = const.tile([1, P], F32, tag="ones_row32", name="ones_row32")
        nc.gpsimd.memset(ones_row32[:], 1.0)
        ones_r = const.tile([1, P], F32R, tag="ones_r", name="ones_r")
        nc.vector.tensor_copy(out=ones_r[:], in_=ones_row32[:])

        colb1_s = const.tile([P, MT5], F32, tag="colb1", name="colb1_s")
        nc.sync.dma_start(colb1_s[:], colb1_d[:, :])
        colb2_s = const.tile([P, MT5], F32, tag="colb2", name="colb2_s")
        nc.sync.dma_start(colb2_s[:], colb2_d[:, :])
        mtc_s = const.tile([P, KT], WN_DT, tag="mtc", name="mtc_s")
        nc.sync.dma_start(mtc_s[:], mtc_d[:, :])

        hw = None
        hin = None
        if HOIST >= 1:
            hw = _load_weights(nc, wpool, w1_t, wv1_t, bv1_d, w2_t, wv2_t,
                               bv2_d, wf_t, bff_d, "H")
        if HOIST >= 2:
            hin = _load_inputs(nc, acts, xT_d, tT_d, "H")
        for it in range(n_iters):
            _body(nc, tc, acts, wpool, io, psum, psum_att, dram_scr,
                  xT_d, tT_d, w1_t, wv1_t, bv1_d, w2_t, wv2_t, bv2_d, wf_t,
                  bff_d, out_d, outm_d, colb1_s, colb2_s, mtc_s, ones_c,
                  ones_r, it, hw, hin)

    nc.compile()
    return nc


def _load_inputs(nc, acts, xT_d, tT_d, it):
    xT = acts.tile([P, KT, NV], AD, tag="sA", name=f"xT_{it}", bufs=IN_BUFS)
    nc.sync.dma_start(xT[:], xT_d.rearrange("p (a n) -> p a n", a=KT))
    tT = acts.tile([P, KT, NV], AD, tag="sB", name=f"tT_{it}", bufs=IN_BUFS)
    nc.sync.dma_start(tT[:], tT_d.rearrange("p (a n) -> p a n", a=KT))
    return xT, tT


def _load_weights(nc, wpool, w1_t, wv1_t, bv1_d, w2_t, wv2_t, bv2_d, wf_t,
                  bff_d, it):
    # hoisted tiles live forever -> each needs its own (tag, bufs=1) slot
    d = {}
    wsrc = {"w1": (w1_t, WT_DT), "wv1": (wv1_t, WN_DT), "w2": (w2_t, WT_DT),
            "wv2": (wv2_t, WN_DT), "wf": (wf_t, WN_DT)}
    for k, (ap, dt) in wsrc.items():
        d[k] = wpool.tile([P, KT, C], dt, tag=f"wh_{k}", name=f"{k}_{it}", bufs=1)
        nc.sync.dma_start(d[k][:], ap[:])
    bsrc = {"bv1": bv1_d, "bv2": bv2_d, "bff": bff_d}
    for k, ap in bsrc.items():
        d[k] = wpool.tile([P, C], F32, tag=f"bh_{k}", name=f"{k}_{it}", bufs=1)
        nc.sync.dma_start(d[k][:], ap[0:1, :].partition_broadcast(P))
    return d


def _body(nc, tc, acts, wpool, io, psum, psum_att, dram_scr,
          xT_d, tT_d, w1_t, wv1_t, bv1_d, w2_t, wv2_t, bv2_d, wf_t,
          bff_d, out_d, outm_d, colb1_s, colb2_s, mtc_s, ones_c, ones_r, it,
          hw=None, hin=None):
    # ---- input DMA (double-buffered slots so the next body prefetches) ----
    if hin is not None:
        xT, tT = hin
    else:
        xT, tT = _load_inputs(nc, acts, xT_d, tT_d, it)

    # ---- z1 = x @ (scale*Wq1@Wk1^T) ----
    if hw is not None:
        w1_s = hw["w1"]
    else:
        w1_s = wpool.tile([P, KT, C], WT_DT, tag="w", name=f"w1_{it}", bufs=W_BUFS)
        nc.sync.dma_start(w1_s[:], w1_t[:])
    z1T = acts.tile([P, KT, NV], AD, tag="sC", name=f"z1T_{it}")
    _proj_t(nc, psum, w1_s, xT, z1T)

    # ---- v1 = x @ Wv1 + bv1 ----
    if hw is not None:
        wv1_s, bv1_bc = hw["wv1"], hw["bv1"]
    else:
        wv1_s = wpool.tile([P, KT, C], WN_DT, tag="w", name=f"wv1_{it}", bufs=W_BUFS)
        nc.sync.dma_start(wv1_s[:], wv1_t[:])
        bv1_bc = wpool.tile([P, C], F32, tag="bbc", name=f"bv1_{it}", bufs=2)
        nc.sync.dma_start(bv1_bc[:], bv1_d[0:1, :].partition_broadcast(P))
    vn = acts.tile([P, MT5, C], VN_DT, tag="sE", name=f"vn_{it}")
    _proj_n(nc, psum, io, xT, wv1_s, bv1_bc, vn)

    # ---- attention 1 (keys = raw x via folding) ----
    o1nT = acts.tile([P, KT, NV], AD, tag="sD", name=f"o1nT_{it}")
    _attention(nc, io, psum, psum_att, z1T, xT, vn, colb1_s, o1nT, ones_c,
               ones_r, f"a1_{it}")

    # ---- z2 = o1n @ (scale*Wq@Wk2^T) ----
    if hw is not None:
        w2_s = hw["w2"]
    else:
        w2_s = wpool.tile([P, KT, C], WT_DT, tag="w", name=f"w2_{it}", bufs=W_BUFS)
        nc.sync.dma_start(w2_s[:], w2_t[:])
    z2_tag = "sF" if hin is not None else "sA"
    z2T = acts.tile([P, KT, NV], AD, tag=z2_tag, name=f"z2T_{it}", bufs=IN_BUFS)
    _proj_t(nc, psum, w2_s, o1nT, z2T)

    # ---- cv = t @ Wv2 + bv2 ----
    if hw is not None:
        wv2_s, bv2_bc = hw["wv2"], hw["bv2"]
    else:
        wv2_s = wpool.tile([P, KT, C], WN_DT, tag="w", name=f"wv2_{it}", bufs=W_BUFS)
        nc.sync.dma_start(wv2_s[:], wv2_t[:])
        bv2_bc = wpool.tile([P, C], F32, tag="bbc", name=f"bv2_{it}", bufs=2)
        nc.sync.dma_start(bv2_bc[:], bv2_d[0:1, :].partition_broadcast(P))
    cvn = acts.tile([P, MT5, C], VN_DT, tag="sE", name=f"cvn_{it}")
    _proj_n(nc, psum, io, tT, wv2_s, bv2_bc, cvn)

    # ---- mean-row part 1: cv_mean = mean_t @ Wv2 + bv2 (masked-row output
    # seed; mean_t comes in column layout so no transpose is needed) ----
    cvm_row = io.tile([1, C], WN_DT, tag="cvm", name=f"cvm_{it}", bufs=2)
    for off, w in VCH:
        ps = psum.tile([P, 512], F32, tag="st", name=f"ps_cvm_{it}_{off}")
        for a in range(KT):
            nc.tensor.matmul(
                ps[0:1, :w],
                mtc_s[:, a : a + 1],
                wv2_s[:, a, off : off + w],
                start=(a == 0),
                stop=(a == KT - 1),
            )
        nc.any.tensor_add(
            out=cvm_row[0:1, off : off + w], in0=ps[0:1, :w], in1=bv2_bc[0:1, off : off + w]
        )
    # row -> column layout via DRAM bounce (off critical path)
    scr = dram_scr.tile([1, C], WN_DT, tag="scr", name=f"scr_{it}", bufs=2)
    nc.sync.dma_start(scr[:], cvm_row[:])
    cvm_col = io.tile([P, KT], WN_DT, tag="cvmc", name=f"cvmc_{it}", bufs=2)
    nc.sync.dma_start(cvm_col[:], scr[0].rearrange("(a p) -> p a", p=P))

    # ---- attention 2 (keys = raw t via folding) ----
    o2nT = acts.tile([P, KT, NV], AD, tag="sC", name=f"o2nT_{it}")
    _attention(nc, io, psum, psum_att, z2T, tT, cvn, colb2_s, o2nT, ones_c,
               ones_r, f"a2_{it}")

    # ---- ffn ----
    if hw is not None:
        wf_s, bf_bc = hw["wf"], hw["bff"]
    else:
        wf_s = wpool.tile([P, KT, C], WN_DT, tag="w", name=f"wf_{it}", bufs=W_BUFS)
        nc.sync.dma_start(wf_s[:], wf_t[:])
        bf_bc = wpool.tile([P, C], F32, tag="bbc", name=f"bff_{it}", bufs=2)
        nc.sync.dma_start(bf_bc[:], bff_d[0:1, :].partition_broadcast(P))

    # mean-row part 2: outm = cv_mean @ Wffn + bff
    outm_row = io.tile([1, C], F32, tag="outm", name=f"outm_{it}", bufs=2)
    for off, w in VCH:
        ps = psum.tile([P, 512], F32, tag="st", name=f"ps_om_{it}_{off}")
        for a in range(KT):
            nc.tensor.matmul(
                ps[0:1, :w],
                cvm_col[:, a : a + 1],
                wf_s[:, a, off : off + w],
                start=(a == 0),
                stop=(a == KT - 1),
            )
        nc.any.tensor_add(
            out=outm_row[0:1, off : off + w], in0=ps[0:1, :w], in1=bf_bc[0:1, off : off + w]
        )
    nc.sync.dma_start(outm_d[:, :], outm_row[:])

    for i in range(MT5):
        pss = []
        for off, w in VCH:
            ps = psum.tile([P, 512], F32, tag="st", name=f"ps_f_{it}_{i}_{off}")
            for a in range(KT):
                nc.tensor.matmul(
                    ps[:, :w],
                    o2nT[:, a, i * P : (i + 1) * P],
                    wf_s[:, a, off : off + w],
                    start=(a == 0),
                    stop=(a == KT - 1),
                )
            pss.append(ps)
        fin = io.tile([P, C], BF, tag="fin", name=f"fin_{it}_{i}", bufs=3)
        for (off, w), ps in zip(VCH, pss):
            nc.any.tensor_add(
                out=fin[:, off : off + w], in0=ps[:, :w], in1=bf_bc[:, off : off + w]
            )
        nc.sync.dma_start(out_d[i * P : (i + 1) * P, :], fin[:])


def _pack(mat_T, dt):
    """[C_in, n] (transposed tensor, contraction on rows) -> [P, KT*n] where
    row p holds tiles a=0..KT-1 contiguously: out[p, a*n+j] = mat_T[a*P+p, j]."""
    n = mat_T.shape[1]
    return np.ascontiguousarray(
        mat_T.reshape(KT, P, n).transpose(1, 0, 2).reshape(P, KT * n)
    ).astype(dt)


def prepare_in_maps(layout_x, text_x, mask, Wqkv, bqkv, Wq, bq, Wkv, bkv,
                    Wffn, bffn):
    """Host-side sharding/layout prep: per-core input maps + scatter metadata."""
    layout_x = np.asarray(layout_x, dtype=np.float32)
    text_x = np.asarray(text_x, dtype=np.float32)
    mask = np.asarray(mask, dtype=np.float32)
    Wqkv = np.asarray(Wqkv, dtype=np.float32)
    bqkv = np.asarray(bqkv, dtype=np.float32).reshape(3 * C)
    Wq = np.asarray(Wq, dtype=np.float32)
    bq = np.asarray(bq, dtype=np.float32).reshape(C)
    Wkv = np.asarray(Wkv, dtype=np.float32)
    bkv = np.asarray(bkv, dtype=np.float32).reshape(2 * C)
    Wffn = np.asarray(Wffn, dtype=np.float32)
    bffn = np.asarray(bffn, dtype=np.float32).reshape(C)

    Wq1, Wk1, Wv1 = Wqkv[:, :C], Wqkv[:, C : 2 * C], Wqkv[:, 2 * C :]
    Wk2, Wv2 = Wkv[:, :C], Wkv[:, C : 2 * C]
    # bilinear weight folds (weight-only preprocessing)
    w1 = _pack(SCALE * (Wq1 @ Wk1.T), _np_of(WT_DT))
    w2 = _pack(SCALE * (Wq @ Wk2.T), _np_of(WT_DT))
    g1 = Wk1 @ bqkv[:C]  # per-key score bias term from bq1 (zero in practice)
    g2 = Wk2 @ bq
    wv1_b = _pack(Wv1, _np_of(WN_DT))
    wv2_b = _pack(Wv2, _np_of(WN_DT))
    wf_b = _pack(Wffn, _np_of(WN_DT))
    bv1 = bqkv[2 * C :].reshape(1, C).astype(np.float32)
    bv2 = bkv[C:].reshape(1, C).astype(np.float32)
    bff = bffn.reshape(1, C).astype(np.float32)

    B = layout_x.shape[0]
    in_maps, metas = [], []
    for b in range(B):
        idx = np.nonzero(mask[b])[0]
        nv = len(idx)
        assert 0 < nv <= NV, f"valid count {nv} outside (0, {NV}]"
        xc = np.zeros((NV, C), np.float32)
        xc[:nv] = layout_x[b][idx]
        tc_ = np.zeros((NV, C), np.float32)
        tc_[:nv] = text_x[b][idx]
        mc = np.zeros(NV, np.float32)
        mc[:nv] = 1.0
        colb1 = (-10000.0 * (1.0 - mc) + SCALE * (xc @ g1)).astype(np.float32)
        colb2 = (-10000.0 * (1.0 - mc) + SCALE * (tc_ @ g2)).astype(np.float32)
        mean_t = text_x[b].mean(axis=0)  # over ALL rows incl. masked
        in_maps.append({
            "xT": _pack(xc.T, _np_of(AD)),
            "tT": _pack(tc_.T, _np_of(AD)),
            "w1": w1, "wv1": wv1_b, "bv1": bv1,
            "w2": w2, "wv2": wv2_b, "bv2": bv2,
            "wf": wf_b, "bff": bff,
            "colb1": np.ascontiguousarray(colb1.reshape(MT5, P).T),
            "colb2": np.ascontiguousarray(colb2.reshape(MT5, P).T),
            "mtc": np.ascontiguousarray(mean_t.reshape(KT, P).T).astype(_np_of(WN_DT)),
        })
        metas.append((idx, nv))
    return in_maps, metas


import os as _os

# Default: bf16 with 256-wide free chunks (best measured on HW); env var is a
# dev-only override for experiments.
set_variant(_os.environ.get("KERNEL_VARIANT", "bf16c"))

_NC_CACHE = None


def _get_nc():
    global _NC_CACHE
    if _NC_CACHE is None:
        _NC_CACHE = build_nc()
    return _NC_CACHE


def kernel(layout_x, text_x, mask, Wqkv, bqkv, Wq, bq, Wkv, bkv, Wffn, bffn):
    in_maps, metas = prepare_in_maps(
        layout_x, text_x, mask, Wqkv, bqkv, Wq, bq, Wkv, bkv, Wffn, bffn
    )
    B = len(in_maps)
    assert B == N_CORES
    nc = _get_nc()
    res = run_bass_kernel_spmd(nc, in_maps, core_ids=list(range(N_CORES)))
    mask = np.asarray(mask, dtype=np.float32)
    out = np.zeros((B, M, C), np.float32)
    for b in range(B):
        idx, nv = metas[b]
        oc = np.asarray(res.results[b]["out"]).astype(np.float32)
        om = np.asarray(res.results[b]["outm"]).astype(np.float32)
        out[b][idx] = oc[:nv]
        out[b][mask[b] == 0] = om[0]
    return out



# revision 5
# speedup vs baseline: 1.6657x; 1.6657x over previous
"""Trainium2 Bass kernel for nn_Attention_kv (dense transformer block).

Sharding: data-parallel over batch B=8 across the 8 NeuronCores -- one batch
element per core, no collectives (host scatters inputs / stacks outputs).

Algorithmic structure (per core, seq M=1024, dim C=768):

1. MASK COMPACTION (host): the pair mask is outer(mask, mask).  Keys with
   mask==0 contribute exactly 0 to every softmax (exp(-10000) underflows),
   and every masked QUERY row's output is the uniform average of ALL value
   rows -- one shared row that only depends on mean(text_x), computed on the
   host (outm = (mean_t @ Wv2 + bv2) @ Wffn + bffn, cheaper than the mean_t
   reduction itself).  Host gathers valid rows, zero-pads to a static NV=544
   (actual max count is 534), and scatters the compacted outputs back.

2. ALGEBRAIC FOLDING (host weight preprocessing, all exact):
   a) scores = scale*(xWq)(xWk)^T = x (scale*Wq Wk^T) x^T: q/k projections
      collapse into ONE projection z = x @ W~.  Per-query score bias terms
      are softmax-invariant (dropped exactly); per-key terms fold into the
      additive key-mask column bias colb (zero when bq==0).
   b) o1 (attn1 output) is ONLY used as cq = o1 @ Wq, so attn1's value
      projection folds with the z2 fold: v1'' = x @ (Wv1 @ (scale*Wq@Wk2^T)).
      Row-normalization commutes with the right-multiply, so attn1 emits z2
      DIRECTLY -- the whole z2 projection stage is eliminated.
   c) merge is ONLY used as merge @ Wffn + bffn, so attn2's value projection
      folds: v2'' = t @ (Wv2@Wffn) + (bv2@Wffn + bffn).  A constant row bias
      passes through softmax averaging exactly (weights sum to 1), so the
      final output is just the normalized attn2 accumulation -- the whole
      FFN stage is eliminated and no final bias add is needed.

3. LAYOUT: attn1 output is accumulated TRANSPOSED (z2T, d on partitions,
   d-outer psum-bank reuse) because attn2's scores consume it as the moving
   operand.  attn2 output is accumulated NATURAL (128-query tiles on psum
   partitions, p-slices stationary): a ones-column appended to v2'' gives
   the softmax denominator for free in the same psum, normalization is a
   per-partition scalar multiply fused into the PSUM->SBUF copyback on the
   scalar engine, and rows DMA straight out.

4. bf16 everywhere off-chip (tolerance 2e-2); PSUM accumulation fp32;
   softmax exp on fp32 scores with the key-mask bias as the activation
   bias operand.

5. Host pre-transposes/packs every tensor as [P, KT*n] so each DMA reads
   ONE contiguous chunk per partition; input DMAs are issued in
   consumption order (w1/xT halves interleaved first) so the first
   projection starts ~3us in.
"""

import sys

sys.path.insert(0, "/opt/trn_rl_repo")

from contextlib import ExitStack

import numpy as np
import ml_dtypes

import concourse.bass as bass
import concourse.mybir as mybir
import concourse.tile as tile
from concourse import bacc
from concourse.bass_utils import run_bass_kernel_spmd

P = 128
M = 1024  # original sequence length per batch element
C = 768  # model dim
KT = C // P  # 6 contraction tiles
SCALE = float(C) ** -0.5
BF16 = ml_dtypes.bfloat16

F32 = mybir.dt.float32
F32R = mybir.dt.float32r
BF = mybir.dt.bfloat16
AF = mybir.ActivationFunctionType

N_CORES = 8

VCH = [(0, 256), (256, 256), (512, 256)]  # natural-projection free chunks of C
VN_COLS = C + 8  # v2 gets a ones column at col C (rowsum augmentation)

# NV (compacted/padded sequence length) is chosen at runtime from the actual
# mask counts (prepare_in_maps -> set_nv); defaults cover the observed inputs.
NV = MT = KW = FCHS = QTS = PWMAX = None


def set_nv(nv_max):
    """Configure the compacted sequence length and derived tilings."""
    global NV, MT, KW, FCHS, QTS, PWMAX
    nv = ((int(nv_max) + 31) // 32) * 32
    assert 0 < nv <= 1024
    NV = nv
    MT = (NV + P - 1) // P
    KW = [min(P, NV - j * P) for j in range(MT)]
    QTS = [(g, min(P, NV - g)) for g in range(0, NV, P)]
    if NV <= 512:
        FCHS = [(0, NV)]
    else:
        split = ((NV // 2 + 64) // P) * P  # balanced split on a 128 boundary
        FCHS = [(0, split), (split, NV - split)]
    PWMAX = max(w for _, w in FCHS)


set_nv(576)

AD = BF  # transposed activations (xT/tT/z1T/z2T)
P_DT = BF  # softmax weights
VN_DT = BF  # natural values
WT_DT = BF  # folded score weights (stationary)
WN_DT = BF  # natural-proj weights
W_BUFS = 4
IN_BUFS = 2


def _np_of(dt):
    return mybir.dt.np(dt)


def _chunk_of(g0, w):
    """(chunk index, local offset) of query range [g0, g0+w) within FCHS."""
    for ci, (off, fw) in enumerate(FCHS):
        if off <= g0 and g0 + w <= off + fw:
            return ci, g0 - off
    raise AssertionError((g0, w))


def _proj_t(nc, psum, w_s, src, dst):
    """dst[:, d, :] ([P, KT, NV] transposed layout) = (src_rows @ W), no bias.

    w_s: [P, KT, C] weight in SBUF (contraction tile a on partitions).
    src: [P, KT, NV] transposed activations (rhs).
    """
    for d in range(KT):
        for off, fw in FCHS:
            ps = psum.tile([P, 512], F32, tag="st", name=f"ps_{dst.name}_{d}_{off}")
            for a in range(KT):
                nc.tensor.matmul(
                    ps[:, :fw],
                    w_s[:, a, d * P : (d + 1) * P],
                    src[:, a, off : off + fw],
                    start=(a == 0),
                    stop=(a == KT - 1),
                )
            nc.any.tensor_copy(out=dst[:, d, off : off + fw], in_=ps[:, :fw])


def _proj_n(nc, psum, src_t, w_s, bias_bc, dst):
    """dst[:, i, :C] ([P, MT, >=C] natural layout) = src @ W + bias."""
    for i in range(MT):
        kw = KW[i]
        pss = []
        for off, w in VCH:
            ps = psum.tile([P, 512], F32, tag="st", name=f"ps_{dst.name}_{i}_{off}")
            for a in range(KT):
                nc.tensor.matmul(
                    ps[:kw, :w],
                    src_t[:, a, i * P : i * P + kw],
                    w_s[:, a, off : off + w],
                    start=(a == 0),
                    stop=(a == KT - 1),
                )
            pss.append(ps)
        for (off, w), ps in zip(VCH, pss):
            nc.any.tensor_add(
                out=dst[:kw, i, off : off + w], in0=ps[:kw, :w], in1=bias_bc[:kw, off : off + w]
            )


def _attention_t(nc, io, psum, psum_att, qT, kT, vn, colb, outT, ones_c, ones_r, label):
    """outT ([P, KT, NV]) = normalized masked attention output^T.

    qT: [P, KT, NV] z-projection (scale already folded in); kT: [P, KT, NV]
    raw transposed keys (folding turned the k-projection into identity);
    vn: [P, MT, C] natural values; colb: [P, MT] additive per-key bias
    (-10000 on masked/padding keys).  Max-free softmax: scores are O(1).

    Two phases per query-chunk: (S) all score chains + exp + rowsum, (O) the
    out^T accumulation looped d-outer so consecutive matmuls accumulate
    into ONE psum bank (bank cycling measured ~+30% per-mm on HW).
    """
    for ci, (off, fw) in enumerate(FCHS):
        # ---- phase S: scores + exp + rowsum ----
        p_tiles = []
        rs = psum_att.tile([P, 512], F32, tag="rs", name=f"rs_{label}_{ci}", bufs=1)
        for j in range(MT):
            kw = KW[j]
            st = psum.tile([P, 512], F32, tag="st", name=f"st_{label}_{ci}_{j}")
            for a in range(KT):
                nc.tensor.matmul(
                    st[:kw, :fw],
                    kT[:, a, j * P : j * P + kw],
                    qT[:, a, off : off + fw],
                    start=(a == 0),
                    stop=(a == KT - 1),
                )
            pj = io.tile([P, PWMAX], P_DT, tag="pp", name=f"p_{label}_{ci}_{j}", bufs=12)
            nc.scalar.activation(pj[:kw, :fw], st[:kw, :fw], AF.Exp, bias=colb[:kw, j : j + 1])
            p_tiles.append(pj)
        for j in range(MT):
            nc.tensor.matmul(
                rs[0:1, :fw],
                ones_c[: KW[j]],
                p_tiles[j][: KW[j], :fw],
                start=(j == 0),
                stop=(j == MT - 1),
            )
        recip = io.tile([1, PWMAX], F32R, tag="recip", name=f"recip_{label}_{ci}", bufs=2)
        with nc.allow_low_precision(reason="f32r recip feeds f32r bcast matmul"):
            nc.vector.reciprocal(recip[:, :fw], rs[0:1, :fw])
        bc = psum_att.tile([P, 512], F32, tag="rs", name=f"bc_{label}_{ci}", bufs=1)
        nc.tensor.matmul(bc[:, :fw], ones_r[:], recip[:, :fw], start=True, stop=True)
        rbc = io.tile([P, PWMAX], F32, tag="rbc", name=f"rbc_{label}_{ci}", bufs=4)
        nc.any.tensor_copy(out=rbc[:, :fw], in_=bc[:, :fw])
        # ---- phase O: out^T accumulation, one bank per d ----
        for d in range(KT):
            po = psum_att.tile([P, 512], F32, tag="po", name=f"po_{label}_{ci}_{d}",
                               bufs=3)
            for j in range(MT):
                kw = KW[j]
                nc.tensor.matmul(
                    po[:, :fw],
                    vn[:kw, j, d * P : (d + 1) * P],
                    p_tiles[j][:kw, :fw],
                    start=(j == 0),
                    stop=(j == MT - 1),
                )
            nc.any.tensor_mul(
                out=outT[:, d, off : off + fw], in0=po[:, :fw], in1=rbc[:, :fw]
            )


def _attention_nat(nc, io, psum, psum_att, qT, kT, vn_aug, colb, out_d, label, it):
    """Natural-layout attention: out rows = diag(1/rowsum) P v'' , DMAd out.

    vn_aug: [P, MT, VN_COLS] natural values with a ones column at col C.
    The last C-chunk's psum is widened by one column: its col 256 is the
    softmax denominator (from the ones column), so no separate rowsum chain
    or broadcast matmul is needed.  Normalization is a per-partition scale
    in the scalar-engine copyback.
    """
    # ---- phase S: scores + exp per query chunk ----
    all_p = []  # per chunk: list of p tiles (keys on partitions)
    for ci, (off, fw) in enumerate(FCHS):
        p_tiles = []
        for j in range(MT):
            kw = KW[j]
            st = psum.tile([P, 512], F32, tag="st", name=f"st_{label}_{ci}_{j}")
            for a in range(KT):
                nc.tensor.matmul(
                    st[:kw, :fw],
                    kT[:, a, j * P : j * P + kw],
                    qT[:, a, off : off + fw],
                    start=(a == 0),
                    stop=(a == KT - 1),
                )
            pj = io.tile([P, PWMAX], P_DT, tag="pp", name=f"p_{label}_{ci}_{j}", bufs=12)
            nc.scalar.activation(pj[:kw, :fw], st[:kw, :fw], AF.Exp, bias=colb[:kw, j : j + 1])
            p_tiles.append(pj)
        all_p.append(p_tiles)
        # ---- phase O for the query tiles inside this chunk ----
        for qi, (g0, qw) in enumerate(QTS):
            cqi, loc = _chunk_of(g0, qw)
            if cqi != ci:
                continue
            pss = []
            # last C-chunk first: it carries the rowsum column
            order = [2, 0, 1]
            for oi in order:
                off_c, w = VCH[oi]
                w_full = w + 1 if oi == 2 else w  # +1 = ones column
                po = psum_att.tile([P, 512], F32, tag="po",
                                   name=f"po_{label}_{qi}_{off_c}", bufs=3)
                for j in range(MT):
                    kw = KW[j]
                    nc.tensor.matmul(
                        po[:qw, :w_full],
                        p_tiles[j][:kw, loc : loc + qw],
                        vn_aug[:kw, j, off_c : off_c + w_full],
                        start=(j == 0),
                        stop=(j == MT - 1),
                    )
                pss.append((oi, off_c, w, po))
            recip = io.tile([P, 1], F32, tag="rcol", name=f"rc_{label}_{qi}", bufs=4)
            po2 = pss[0][3]
            nc.vector.reciprocal(recip[:qw], po2[:qw, 256:257])
            fin = io.tile([P, C], BF, tag="fin", name=f"fin_{label}_{qi}", bufs=3)
            for oi, off_c, w, po in pss:
                nc.scalar.activation(
                    fin[:qw, off_c : off_c + w], po[:qw, :w], AF.Copy, scale=recip[:qw]
                )
            nc.sync.dma_start(out_d[g0 : g0 + qw, :], fin[:qw])


def build_nc(n_iters=1):
    nc = bacc.Bacc(trn_type="TRN2", target_bir_lowering=False, debug=False)

    # weights/inputs come pre-packed [P, KT*n] on the host so every partition
    # reads ONE contiguous chunk (128 descriptors per DMA instead of 768)
    xT_d = nc.dram_tensor("xT", [P, KT * NV], AD, kind="ExternalInput").ap()
    tT_d = nc.dram_tensor("tT", [P, KT * NV], AD, kind="ExternalInput").ap()
    w1_d = nc.dram_tensor("w1", [P, KT * C], WT_DT, kind="ExternalInput").ap()
    w2v_d = nc.dram_tensor("w2v", [P, KT * C], WN_DT, kind="ExternalInput").ap()
    b2v_d = nc.dram_tensor("b2v", [1, C], F32, kind="ExternalInput").ap()
    wvf_d = nc.dram_tensor("wvf", [P, KT * C], WN_DT, kind="ExternalInput").ap()
    bvf_d = nc.dram_tensor("bvf", [1, C], F32, kind="ExternalInput").ap()
    colb1_d = nc.dram_tensor("colb1", [P, MT], F32, kind="ExternalInput").ap()
    colb2_d = nc.dram_tensor("colb2", [P, MT], F32, kind="ExternalInput").ap()
    out_d = nc.dram_tensor("out", [NV, C], BF, kind="ExternalOutput").ap()

    w1_t = w1_d.rearrange("p (a n) -> p a n", a=KT)
    w2v_t = w2v_d.rearrange("p (a n) -> p a n", a=KT)
    wvf_t = wvf_d.rearrange("p (a n) -> p a n", a=KT)

    with tile.TileContext(nc) as tc, ExitStack() as ctx:
        const = ctx.enter_context(tc.tile_pool(name="const", bufs=1))
        acts = ctx.enter_context(tc.tile_pool(name="acts", bufs=1))
        wpool = ctx.enter_context(tc.tile_pool(name="wpool", bufs=1))
        io = ctx.enter_context(tc.tile_pool(name="io", bufs=1))
        psum = ctx.enter_context(tc.tile_pool(name="psum_main", bufs=4, space="PSUM"))
        psum_att = ctx.enter_context(tc.tile_pool(name="psum_att", bufs=3, space="PSUM"))

        # ---- constants ----
        ones32 = const.tile([P, 1], F32, tag="ones32", name="ones32")
        nc.gpsimd.memset(ones32[:], 1.0)
        ones_c = const.tile([P, 1], P_DT, tag="ones_c", name="ones_c")
        nc.vector.tensor_copy(out=ones_c[:], in_=ones32[:])
        ones_row32 = const.tile([1, P], F32, tag="ones_row32", name="ones_row32")
        nc.gpsimd.memset(ones_row32[:], 1.0)
        ones_r = const.tile([1, P], F32R, tag="ones_r", name="ones_r")
        nc.vector.tensor_copy(out=ones_r[:], in_=ones_row32[:])

        colb1_s = const.tile([P, MT], F32, tag="colb1", name="colb1_s")
        nc.sync.dma_start(colb1_s[:], colb1_d[:, :])
        colb2_s = const.tile([P, MT], F32, tag="colb2", name="colb2_s")
        nc.sync.dma_start(colb2_s[:], colb2_d[:, :])

        for it in range(n_iters):
            _body(nc, tc, acts, wpool, io, psum, psum_att,
                  xT_d, tT_d, w1_t, w2v_t, b2v_d, wvf_t, bvf_d, out_d,
                  colb1_s, colb2_s, ones_c, ones_r, it)

    nc.compile()
    return nc


def _body(nc, tc, acts, wpool, io, psum, psum_att,
          xT_d, tT_d, w1_t, w2v_t, b2v_d, wvf_t, bvf_d, out_d,
          colb1_s, colb2_s, ones_c, ones_r, it):
    # ---- DMA in consumption order; w1/xT halves interleaved so the first
    # projection chain can start after ~half the critical input bytes ----
    H = KT // 2
    w1_s = wpool.tile([P, KT, C], WT_DT, tag="w", name=f"w1_{it}", bufs=W_BUFS)
    xT = acts.tile([P, KT, NV], AD, tag="sA", name=f"xT_{it}", bufs=IN_BUFS)
    nc.sync.dma_start(w1_s[:, :H], w1_t[:, :H])
    nc.sync.dma_start(xT[:, :H], xT_d.rearrange("p (a n) -> p a n", a=KT)[:, :H])
    nc.sync.dma_start(w1_s[:, H:], w1_t[:, H:])
    nc.sync.dma_start(xT[:, H:], xT_d.rearrange("p (a n) -> p a n", a=KT)[:, H:])
    w2v_s = wpool.tile([P, KT, C], WN_DT, tag="w", name=f"w2v_{it}", bufs=W_BUFS)
    nc.sync.dma_start(w2v_s[:], w2v_t[:])
    b2v_bc = wpool.tile([P, C], F32, tag="bbc", name=f"b2v_{it}", bufs=2)
    nc.sync.dma_start(b2v_bc[:], b2v_d[0:1, :].partition_broadcast(P))
    tT = acts.tile([P, KT, NV], AD, tag="sB", name=f"tT_{it}", bufs=IN_BUFS)
    nc.sync.dma_start(tT[:], tT_d.rearrange("p (a n) -> p a n", a=KT))
    wvf_s = wpool.tile([P, KT, C], WN_DT, tag="w", name=f"wvf_{it}", bufs=W_BUFS)
    nc.sync.dma_start(wvf_s[:], wvf_t[:])
    bvf_bc = wpool.tile([P, C], F32, tag="bbc", name=f"bvf_{it}", bufs=2)
    nc.sync.dma_start(bvf_bc[:], bvf_d[0:1, :].partition_broadcast(P))

    # ---- z1 = x @ (scale*Wq1@Wk1^T) ----
    z1T = acts.tile([P, KT, NV], AD, tag="sC", name=f"z1T_{it}")
    _proj_t(nc, psum, w1_s, xT, z1T)

    # ---- v1'' = x @ (Wv1 @ scale*Wq@Wk2^T) + b2v  (attn1 values == z2 seed) ----
    v1n = acts.tile([P, MT, C], VN_DT, tag="sE", name=f"v1n_{it}")
    _proj_n(nc, psum, xT, w2v_s, b2v_bc, v1n)

    # ---- attention 1 (keys = raw x via folding) -> z2T directly ----
    z2T = acts.tile([P, KT, NV], AD, tag="sD", name=f"z2T_{it}", bufs=IN_BUFS)
    _attention_t(nc, io, psum, psum_att, z1T, xT, v1n, colb1_s, z2T, ones_c,
                 ones_r, f"a1_{it}")

    # ---- v2'' = t @ (Wv2@Wffn) + (bv2@Wffn + bffn), ones col at C ----
    v2n = acts.tile([P, MT, VN_COLS], VN_DT, tag="sF", name=f"v2n_{it}")
    for j in range(MT):
        nc.vector.memset(v2n[: KW[j], j, C : C + 1], 1.0)
    _proj_n(nc, psum, tT, wvf_s, bvf_bc, v2n)

    # ---- attention 2 (keys = raw t via folding), natural out -> DMA ----
    _attention_nat(nc, io, psum, psum_att, z2T, tT, v2n, colb2_s, out_d,
                   f"a2_{it}", it)


def _pack(mat_T, dt):
    """[C_in, n] (transposed tensor, contraction on rows) -> [P, KT*n] where
    row p holds tiles a=0..KT-1 contiguously: out[p, a*n+j] = mat_T[a*P+p, j]."""
    n = mat_T.shape[1]
    return np.ascontiguousarray(
        mat_T.reshape(KT, P, n).transpose(1, 0, 2).reshape(P, KT * n)
    ).astype(dt)


def prepare_in_maps(layout_x, text_x, mask, Wqkv, bqkv, Wq, bq, Wkv, bkv,
                    Wffn, bffn):
    """Host-side sharding/layout prep: per-core input maps + scatter metadata."""
    layout_x = np.asarray(layout_x, dtype=np.float32)
    text_x = np.asarray(text_x, dtype=np.float32)
    mask = np.asarray(mask, dtype=np.float32)
    Wqkv = np.asarray(Wqkv, dtype=np.float64)
    bqkv = np.asarray(bqkv, dtype=np.float64).reshape(3 * C)
    Wq = np.asarray(Wq, dtype=np.float64)
    bq = np.asarray(bq, dtype=np.float64).reshape(C)
    Wkv = np.asarray(Wkv, dtype=np.float64)
    bkv = np.asarray(bkv, dtype=np.float64).reshape(2 * C)
    Wffn = np.asarray(Wffn, dtype=np.float64)
    bffn = np.asarray(bffn, dtype=np.float64).reshape(C)

    Wq1, Wk1, Wv1 = Wqkv[:, :C], Wqkv[:, C : 2 * C], Wqkv[:, 2 * C :]
    Wk2, Wv2 = Wkv[:, :C], Wkv[:, C : 2 * C]
    bv1, bv2 = bqkv[2 * C :], bkv[C:]
    # exact algebraic folds (host, fp64)
    W1f = SCALE * (Wq1 @ Wk1.T)  # z1 fold
    W2t = SCALE * (Wq @ Wk2.T)  # q2/k2 fold
    W2pp = Wv1 @ W2t  # v1 fold: attn1 emits z2 directly
    b2pp = bv1 @ W2t
    Wvf = Wv2 @ Wffn  # v2 fold: attn2 emits the final output
    bvf = bv2 @ Wffn + bffn
    g1 = Wk1 @ bqkv[:C]  # per-key score bias from bq1 (zero in practice)
    g2 = Wk2 @ bq

    w1 = _pack(W1f, _np_of(WT_DT))
    w2v_b = _pack(W2pp, _np_of(WN_DT))
    wvf_b = _pack(Wvf, _np_of(WN_DT))
    b2v = b2pp.reshape(1, C).astype(np.float32)
    bvf_r = bvf.reshape(1, C).astype(np.float32)

    B = layout_x.shape[0]
    counts = [int((mask[b] != 0).sum()) for b in range(B)]
    set_nv(max(max(counts), 1))
    in_maps, metas = [], []
    for b in range(B):
        idx = np.nonzero(mask[b])[0]
        nv = len(idx)
        assert 0 < nv <= NV, f"valid count {nv} outside (0, {NV}]"
        xc = np.zeros((NV, C), np.float32)
        xc[:nv] = layout_x[b][idx]
        tc_ = np.zeros((NV, C), np.float32)
        tc_[:nv] = text_x[b][idx]
        mc = np.zeros(MT * P, np.float32)
        mc[:nv] = 1.0
        colb1 = -10000.0 * (1.0 - mc)
        colb1[:NV] += SCALE * (xc @ g1)
        colb2 = -10000.0 * (1.0 - mc)
        colb2[:NV] += SCALE * (tc_ @ g2)
        mean_t = text_x[b].astype(np.float64).mean(axis=0)  # over ALL rows
        outm = ((mean_t @ Wv2 + bv2) @ Wffn + bffn).astype(np.float32)
        in_maps.append({
            "xT": _pack(xc.T, _np_of(AD)),
            "tT": _pack(tc_.T, _np_of(AD)),
            "w1": w1, "w2v": w2v_b, "b2v": b2v,
            "wvf": wvf_b, "bvf": bvf_r,
            "colb1": np.ascontiguousarray(
                colb1.astype(np.float32).reshape(MT, P).T),
            "colb2": np.ascontiguousarray(
                colb2.astype(np.float32).reshape(MT, P).T),
        })
        metas.append((idx, nv, outm))
    return in_maps, metas


_NC_CACHE = {}


def _get_nc():
    if NV not in _NC_CACHE:
        _NC_CACHE[NV] = build_nc()
    return _NC_CACHE[NV]


def kernel(layout_x, text_x, mask, Wqkv, bqkv, Wq, bq, Wkv, bkv, Wffn, bffn):
    in_maps, metas = prepare_in_maps(
        layout_x, text_x, mask, Wqkv, bqkv, Wq, bq, Wkv, bkv, Wffn, bffn
    )
    B = len(in_maps)
    assert B == N_CORES
    nc = _get_nc()
    res = run_bass_kernel_spmd(nc, in_maps, core_ids=list(range(N_CORES)))
    mask = np.asarray(mask, dtype=np.float32)
    out = np.zeros((B, M, C), np.float32)
    for b in range(B):
        idx, nv, outm = metas[b]
        oc = np.asarray(res.results[b]["out"]).astype(np.float32)
        out[b][idx] = oc[:nv]
        out[b][mask[b] == 0] = outm
    return out


# revision 15
# speedup vs baseline: 2.3639x; 1.4192x over previous
"""Trainium2 Bass kernel for nn_Attention_kv (dense transformer block).

Sharding: data-parallel over batch B=8 across the 8 NeuronCores -- one batch
element per core, no collectives (host scatters inputs / stacks outputs).

Algorithmic structure (per core, seq M=1024, dim C=768):

1. MASK COMPACTION (host): the pair mask is outer(mask, mask).  Keys with
   mask==0 contribute exactly 0 to every softmax (exp(-10000) underflows),
   and every masked QUERY row's output is the uniform average of ALL value
   rows -- one shared row that only depends on mean(text_x), computed on the
   host (outm = (mean_t @ Wv2 + bv2) @ Wffn + bffn, cheaper than the mean_t
   reduction itself).  Host gathers valid rows, zero-pads to a static NV=544
   (actual max count is 534), and scatters the compacted outputs back.

2. ALGEBRAIC FOLDING (host weight preprocessing, all exact):
   a) scores = scale*(xWq)(xWk)^T = x (scale*Wq Wk^T) x^T: q/k projections
      collapse into ONE projection z = x @ W~.  Per-query score bias terms
      are softmax-invariant (dropped exactly); per-key terms fold into the
      additive key-mask column bias colb (zero when bq==0).
   b) o1 (attn1 output) is ONLY used as cq = o1 @ Wq, so attn1's value
      projection folds with the z2 fold: v1'' = x @ (Wv1 @ (scale*Wq@Wk2^T)).
      Row-normalization commutes with the right-multiply, so attn1 emits z2
      DIRECTLY -- the whole z2 projection stage is eliminated.
   c) merge is ONLY used as merge @ Wffn + bffn, so attn2's value projection
      folds: v2'' = t @ (Wv2@Wffn) + (bv2@Wffn + bffn).  A constant row bias
      passes through softmax averaging exactly (weights sum to 1), so the
      final output is just the normalized attn2 accumulation -- the whole
      FFN stage is eliminated and no final bias add is needed.

3. LAYOUT: attn1 output is accumulated TRANSPOSED (z2T, d on partitions,
   d-outer psum-bank reuse) because attn2's scores consume it as the moving
   operand.  attn2 output is accumulated NATURAL (128-query tiles on psum
   partitions, p-slices stationary): a ones-column appended to v2'' gives
   the softmax denominator for free in the same psum, normalization is a
   per-partition scalar multiply fused into the PSUM->SBUF copyback on the
   scalar engine, and rows DMA straight out.

4. bf16 everywhere off-chip (tolerance 2e-2); PSUM accumulation fp32;
   softmax exp on fp32 scores with the key-mask bias as the activation
   bias operand.

5. Host pre-transposes/packs every tensor as [P, KT*n] so each DMA reads
   ONE contiguous chunk per partition; input DMAs are issued in
   consumption order (w1/xT halves interleaved first) so the first
   projection starts ~3us in.
"""

import sys

sys.path.insert(0, "/opt/trn_rl_repo")

from contextlib import ExitStack

import numpy as np
import ml_dtypes

import concourse.bass as bass
import concourse.mybir as mybir
import concourse.tile as tile
from concourse import bacc
from concourse.bass_utils import run_bass_kernel_spmd

P = 128
M = 1024  # original sequence length per batch element
C = 768  # model dim
KT = C // P  # 6 contraction tiles
SCALE = float(C) ** -0.5
BF16 = ml_dtypes.bfloat16

F32 = mybir.dt.float32
F32R = mybir.dt.float32r
BF = mybir.dt.bfloat16
AF = mybir.ActivationFunctionType

N_CORES = 8

VCH = [(0, 512), (512, 256)]  # natural-projection free chunks of C
VCHA = [(512, 257), (0, 512)]  # attn2-out chunks: rowsum-augmented one first
VN_COLS = C + 8  # v2 gets a ones column at col C (rowsum augmentation)
N_WARM = 18  # dummy matmuls that keep/start the PE HAM-warm during input DMA

# NV (compacted/padded sequence length) is chosen at runtime from the actual
# mask counts (prepare_in_maps -> set_nv); defaults cover the observed inputs.
NV = MT = KW = FCHS = QTS = PWMAX = None


def set_nv(nv_max):
    """Configure the compacted sequence length and derived tilings."""
    global NV, MT, KW, FCHS, QTS, PWMAX
    nv = ((int(nv_max) + 3) // 4) * 4  # pad only to a multiple of 4
    assert 0 < nv <= 1024
    NV = nv
    MT = (NV + P - 1) // P
    KW = [min(P, NV - j * P) for j in range(MT)]
    QTS = [(g, min(P, NV - g)) for g in range(0, NV, P)]
    if NV <= 512:
        FCHS = [(0, NV)]
    else:
        split = ((NV // 2 + 64) // P) * P  # balanced split on a 128 boundary
        FCHS = [(0, split), (split, NV - split)]
    PWMAX = max(w for _, w in FCHS)


set_nv(576)

AD = BF  # transposed activations (xT/tT/z1T/z2T)
P_DT = BF  # softmax weights
VN_DT = BF  # natural values
WT_DT = mybir.dt.float8e4  # folded score weight w1 (stationary; tiny entries)
W1SCALE = 32.0  # host pre-scale lifting w1 out of fp8 subnormals; exp undoes it
WN_DT = BF  # natural-proj weights
W_BUFS = 4
IN_BUFS = 2


def _np_of(dt):
    return mybir.dt.np(dt)


def _chunk_of(g0, w):
    """(chunk index, local offset) of query range [g0, g0+w) within FCHS."""
    for ci, (off, fw) in enumerate(FCHS):
        if off <= g0 and g0 + w <= off + fw:
            return ci, g0 - off
    raise AssertionError((g0, w))


def _proj_t(nc, psum, w_s, src, dst):
    """dst[:, d, :] ([P, KT, NV] transposed layout) = (src_rows @ W), no bias.

    w_s: [P, KT, C] weight in SBUF (contraction tile a on partitions).
    src: [P, KT, NV] transposed activations (rhs).
    """
    for d in range(KT):
        for off, fw in FCHS:
            ps = psum.tile([P, 512], F32, tag="st", name=f"ps_{dst.name}_{d}_{off}")
            for a in range(KT):
                nc.tensor.matmul(
                    ps[:, :fw],
                    w_s[:, a, d * P : (d + 1) * P],
                    src[:, a, off : off + fw],
                    start=(a == 0),
                    stop=(a == KT - 1),
                )
            nc.any.tensor_copy(out=dst[:, d, off : off + fw], in_=ps[:, :fw])


def _proj_n(nc, psum, src_t, w_s, bias_bc, dst):
    """dst[:, i, :C] ([P, MT, >=C] natural layout) = src @ W + bias."""
    for i in range(MT):
        kw = KW[i]
        pss = []
        for off, w in VCH:
            ps = psum.tile([P, 512], F32, tag="st", name=f"ps_{dst.name}_{i}_{off}")
            for a in range(KT):
                nc.tensor.matmul(
                    ps[:kw, :w],
                    src_t[:, a, i * P : i * P + kw],
                    w_s[:, a, off : off + w],
                    start=(a == 0),
                    stop=(a == KT - 1),
                )
            pss.append(ps)
        for (off, w), ps in zip(VCH, pss):
            nc.any.tensor_add(
                out=dst[:kw, i, off : off + w], in0=ps[:kw, :w], in1=bias_bc[:kw, off : off + w]
            )


def _attention_t(nc, io, psum, psum_att, qT, kT, vn, colb, outT, ones_c, ones_r, label,
                 escale=1.0):
    """outT ([P, KT, NV]) = normalized masked attention output^T.

    qT: [P, KT, NV] z-projection (scale already folded in); kT: [P, KT, NV]
    raw transposed keys (folding turned the k-projection into identity);
    vn: [P, MT, C] natural values; colb: [P, MT] additive per-key bias
    (-10000 on masked/padding keys).  Max-free softmax: scores are O(1).

    Two phases per query-chunk: (S) all score chains + exp + rowsum, (O) the
    out^T accumulation looped d-outer so consecutive matmuls accumulate
    into ONE psum bank (bank cycling measured ~+30% per-mm on HW).
    """
    for ci, (off, fw) in enumerate(FCHS):
        # ---- phase S: scores + exp + rowsum ----
        p_tiles = []
        rs = psum_att.tile([P, 512], F32, tag="rs", name=f"rs_{label}_{ci}", bufs=1)
        for j in range(MT):
            kw = KW[j]
            st = psum.tile([P, 512], F32, tag="st", name=f"st_{label}_{ci}_{j}")
            for a in range(KT):
                nc.tensor.matmul(
                    st[:kw, :fw],
                    kT[:, a, j * P : j * P + kw],
                    qT[:, a, off : off + fw],
                    start=(a == 0),
                    stop=(a == KT - 1),
                )
            pj = io.tile([P, PWMAX], P_DT, tag="pp", name=f"p_{label}_{ci}_{j}", bufs=12)
            nc.scalar.activation(pj[:kw, :fw], st[:kw, :fw], AF.Exp,
                                 bias=colb[:kw, j : j + 1], scale=escale)
            p_tiles.append(pj)
        for j in range(MT):
            nc.tensor.matmul(
                rs[0:1, :fw],
                ones_c[: KW[j]],
                p_tiles[j][: KW[j], :fw],
                start=(j == 0),
                stop=(j == MT - 1),
            )
        recip = io.tile([1, PWMAX], F32R, tag="recip", name=f"recip_{label}_{ci}", bufs=2)
        with nc.allow_low_precision(reason="f32r recip feeds f32r bcast matmul"):
            nc.vector.reciprocal(recip[:, :fw], rs[0:1, :fw])
        bc = psum_att.tile([P, 512], F32, tag="rs", name=f"bc_{label}_{ci}", bufs=1)
        nc.tensor.matmul(bc[:, :fw], ones_r[:], recip[:, :fw], start=True, stop=True)
        rbc = io.tile([P, PWMAX], F32, tag="rbc", name=f"rbc_{label}_{ci}", bufs=4)
        nc.any.tensor_copy(out=rbc[:, :fw], in_=bc[:, :fw])
        # ---- phase O: out^T accumulation, one bank per d ----
        for d in range(KT):
            po = psum_att.tile([P, 512], F32, tag="po", name=f"po_{label}_{ci}_{d}",
                               bufs=3)
            for j in range(MT):
                kw = KW[j]
                nc.tensor.matmul(
                    po[:, :fw],
                    vn[:kw, j, d * P : (d + 1) * P],
                    p_tiles[j][:kw, :fw],
                    start=(j == 0),
                    stop=(j == MT - 1),
                )
            nc.any.tensor_mul(
                out=outT[:, d, off : off + fw], in0=po[:, :fw], in1=rbc[:, :fw]
            )


def _attention_nat(nc, io, psum, psum_att, qT, kT, vn_aug, colb, out_d, label, it):
    """Natural-layout attention: out rows = diag(1/rowsum) P v'' , DMAd out.

    vn_aug: [P, MT, VN_COLS] natural values with a ones column at col C.
    The last C-chunk's psum is widened by one column: its col 256 is the
    softmax denominator (from the ones column), so no separate rowsum chain
    or broadcast matmul is needed.  Normalization is a per-partition scale
    in the scalar-engine copyback.
    """
    # ---- phase S: scores + exp per query chunk ----
    all_p = []  # per chunk: list of p tiles (keys on partitions)
    for ci, (off, fw) in enumerate(FCHS):
        p_tiles = []
        for j in range(MT):
            kw = KW[j]
            st = psum.tile([P, 512], F32, tag="st", name=f"st_{label}_{ci}_{j}")
            for a in range(KT):
                nc.tensor.matmul(
                    st[:kw, :fw],
                    kT[:, a, j * P : j * P + kw],
                    qT[:, a, off : off + fw],
                    start=(a == 0),
                    stop=(a == KT - 1),
                )
            pj = io.tile([P, PWMAX], P_DT, tag="pp", name=f"p_{label}_{ci}_{j}", bufs=12)
            nc.scalar.activation(pj[:kw, :fw], st[:kw, :fw], AF.Exp, bias=colb[:kw, j : j + 1])
            p_tiles.append(pj)
        all_p.append(p_tiles)
        # ---- phase O for the query tiles inside this chunk ----
        for qi, (g0, qw) in enumerate(QTS):
            cqi, loc = _chunk_of(g0, qw)
            if cqi != ci:
                continue
            pss = []
            # rowsum-augmented C-chunk first: it carries the denominator
            for off_c, w_full in VCHA:
                w = min(w_full, C - off_c)  # data width (w_full may add ones col)
                po = psum_att.tile([P, 512], F32, tag="po",
                                   name=f"po_{label}_{qi}_{off_c}", bufs=3)
                for j in range(MT):
                    kw = KW[j]
                    nc.tensor.matmul(
                        po[:qw, :w_full],
                        p_tiles[j][:kw, loc : loc + qw],
                        vn_aug[:kw, j, off_c : off_c + w_full],
                        start=(j == 0),
                        stop=(j == MT - 1),
                    )
                pss.append((off_c, w, po))
            recip = io.tile([P, 1], F32, tag="rcol", name=f"rc_{label}_{qi}", bufs=4)
            po2 = pss[0][2]
            nc.vector.reciprocal(recip[:qw], po2[:qw, C - VCHA[0][0] : C - VCHA[0][0] + 1])
            fin = io.tile([P, C], BF, tag="fin", name=f"fin_{label}_{qi}", bufs=3)
            for off_c, w, po in pss:
                nc.scalar.activation(
                    fin[:qw, off_c : off_c + w], po[:qw, :w], AF.Copy, scale=recip[:qw]
                )
            nc.sync.dma_start(out_d[g0 : g0 + qw, :], fin[:qw])


def build_nc(n_iters=1):
    nc = bacc.Bacc(trn_type="TRN2", target_bir_lowering=False, debug=False)

    # weights/inputs come pre-packed [P, KT*n] on the host so every partition
    # reads ONE contiguous chunk (128 descriptors per DMA instead of 768)
    xT_d = nc.dram_tensor("xT", [P, KT * NV], AD, kind="ExternalInput").ap()
    tT_d = nc.dram_tensor("tT", [P, KT * NV], AD, kind="ExternalInput").ap()
    w1_d = nc.dram_tensor("w1", [P, KT * C], WT_DT, kind="ExternalInput").ap()
    w2v_d = nc.dram_tensor("w2v", [P, KT * C], WN_DT, kind="ExternalInput").ap()
    b2v_d = nc.dram_tensor("b2v", [1, C], F32, kind="ExternalInput").ap()
    wvf_d = nc.dram_tensor("wvf", [P, KT * C], WN_DT, kind="ExternalInput").ap()
    bvf_d = nc.dram_tensor("bvf", [1, C], F32, kind="ExternalInput").ap()
    colb1_d = nc.dram_tensor("colb1", [P, MT], F32, kind="ExternalInput").ap()
    colb2_d = nc.dram_tensor("colb2", [P, MT], F32, kind="ExternalInput").ap()
    out_d = nc.dram_tensor("out", [NV, C], BF, kind="ExternalOutput").ap()

    w1_t = w1_d.rearrange("p (a n) -> p a n", a=KT)
    w2v_t = w2v_d.rearrange("p (a n) -> p a n", a=KT)
    wvf_t = wvf_d.rearrange("p (a n) -> p a n", a=KT)

    with tile.TileContext(nc) as tc, ExitStack() as ctx:
        const = ctx.enter_context(tc.tile_pool(name="const", bufs=1))
        acts = ctx.enter_context(tc.tile_pool(name="acts", bufs=1))
        wpool = ctx.enter_context(tc.tile_pool(name="wpool", bufs=1))
        io = ctx.enter_context(tc.tile_pool(name="io", bufs=1))
        psum = ctx.enter_context(tc.tile_pool(name="psum_main", bufs=4, space="PSUM"))
        psum_att = ctx.enter_context(tc.tile_pool(name="psum_att", bufs=3, space="PSUM"))

        # ---- constants (no DMA here: input DMAs must hit the queue first) ----
        warm = const.tile([P, 256], BF, tag="warm", name="warm")
        nc.gpsimd.memset(warm[:], 0.0)
        ones32 = const.tile([P, 1], F32, tag="ones32", name="ones32")
        nc.gpsimd.memset(ones32[:], 1.0)
        ones_c = const.tile([P, 1], P_DT, tag="ones_c", name="ones_c")
        nc.vector.tensor_copy(out=ones_c[:], in_=ones32[:])
        ones_row32 = const.tile([1, P], F32, tag="ones_row32", name="ones_row32")
        nc.gpsimd.memset(ones_row32[:], 1.0)
        ones_r = const.tile([1, P], F32R, tag="ones_r", name="ones_r")
        nc.vector.tensor_copy(out=ones_r[:], in_=ones_row32[:])

        colb1_s = const.tile([P, MT], F32, tag="colb1", name="colb1_s")
        colb2_s = const.tile([P, MT], F32, tag="colb2", name="colb2_s")

        # PE warm-up: harmless matmuls during the input-DMA head keep the HAM
        # activity window busy so the first real matmuls run at full clock.
        for wi in range(N_WARM):
            wp = psum.tile([P, 512], F32, tag="st", name=f"warmps_{wi}")
            nc.tensor.matmul(wp[:, :256], warm[:, :P], warm[:],
                             start=True, stop=True)

        for it in range(n_iters):
            _body(nc, tc, acts, wpool, io, psum, psum_att,
                  xT_d, tT_d, w1_t, w2v_t, b2v_d, wvf_t, bvf_d, out_d,
                  colb1_s, colb2_s, colb1_d if it == 0 else None,
                  colb2_d if it == 0 else None, ones_c, ones_r, it)

    nc.compile()
    return nc


def _body(nc, tc, acts, wpool, io, psum, psum_att,
          xT_d, tT_d, w1_t, w2v_t, b2v_d, wvf_t, bvf_d, out_d,
          colb1_s, colb2_s, colb1_d, colb2_d, ones_c, ones_r, it):
    # ---- DMA in consumption order; w1/xT interleaved in thirds so the first
    # projection chains can start after ~1/3 of the critical input bytes ----
    T3 = KT // 3
    w1_s = wpool.tile([P, KT, C], WT_DT, tag="w", name=f"w1_{it}", bufs=W_BUFS)
    xT = acts.tile([P, KT, NV], AD, tag="sA", name=f"xT_{it}", bufs=IN_BUFS)
    xT_r = xT_d.rearrange("p (a n) -> p a n", a=KT)
    for h in range(0, KT, T3):
        nc.sync.dma_start(w1_s[:, h : h + T3], w1_t[:, h : h + T3])
        nc.sync.dma_start(xT[:, h : h + T3], xT_r[:, h : h + T3])
    w2v_s = wpool.tile([P, KT, C], WN_DT, tag="w", name=f"w2v_{it}", bufs=W_BUFS)
    H = KT // 2
    for h in range(0, KT, T3):
        nc.sync.dma_start(w2v_s[:, h : h + T3], w2v_t[:, h : h + T3])
    b2v_bc = wpool.tile([P, C], F32, tag="bbc", name=f"b2v_{it}", bufs=2)
    nc.sync.dma_start(b2v_bc[:], b2v_d[0:1, :].partition_broadcast(P))
    if colb1_d is not None:
        nc.sync.dma_start(colb1_s[:], colb1_d[:, :])
    tT = acts.tile([P, KT, NV], AD, tag="sB", name=f"tT_{it}", bufs=IN_BUFS)
    nc.sync.dma_start(tT[:, :H], tT_d.rearrange("p (a n) -> p a n", a=KT)[:, :H])
    nc.sync.dma_start(tT[:, H:], tT_d.rearrange("p (a n) -> p a n", a=KT)[:, H:])
    wvf_s = wpool.tile([P, KT, C], WN_DT, tag="w", name=f"wvf_{it}", bufs=W_BUFS)
    nc.sync.dma_start(wvf_s[:], wvf_t[:])
    bvf_bc = wpool.tile([P, C], F32, tag="bbc", name=f"bvf_{it}", bufs=2)
    nc.sync.dma_start(bvf_bc[:], bvf_d[0:1, :].partition_broadcast(P))
    if colb2_d is not None:
        nc.sync.dma_start(colb2_s[:], colb2_d[:, :])

    # ---- z1 = x @ (scale*Wq1@Wk1^T) ----
    z1T = acts.tile([P, KT, NV], AD, tag="sC", name=f"z1T_{it}")
    _proj_t(nc, psum, w1_s, xT, z1T)

    # ---- v1'' = x @ (Wv1 @ scale*Wq@Wk2^T) + b2v  (attn1 values == z2 seed) ----
    v1n = acts.tile([P, MT, C], VN_DT, tag="sE", name=f"v1n_{it}")
    _proj_n(nc, psum, xT, w2v_s, b2v_bc, v1n)

    # ---- attention 1 (keys = raw x via folding) -> z2T directly ----
    z2T = acts.tile([P, KT, NV], AD, tag="sD", name=f"z2T_{it}", bufs=IN_BUFS)
    _attention_t(nc, io, psum, psum_att, z1T, xT, v1n, colb1_s, z2T, ones_c,
                 ones_r, f"a1_{it}", escale=1.0 / W1SCALE)

    # ---- v2'' = t @ (Wv2@Wffn) + (bv2@Wffn + bffn), ones col at C ----
    v2n = acts.tile([P, MT, VN_COLS], VN_DT, tag="sF", name=f"v2n_{it}")
    for j in range(MT):
        nc.vector.memset(v2n[: KW[j], j, C : C + 1], 1.0)
    _proj_n(nc, psum, tT, wvf_s, bvf_bc, v2n)

    # ---- attention 2 (keys = raw t via folding), natural out -> DMA ----
    _attention_nat(nc, io, psum, psum_att, z2T, tT, v2n, colb2_s, out_d,
                   f"a2_{it}", it)


def _pack(mat_T, dt):
    """[C_in, n] (transposed tensor, contraction on rows) -> [P, KT*n] where
    row p holds tiles a=0..KT-1 contiguously: out[p, a*n+j] = mat_T[a*P+p, j]."""
    n = mat_T.shape[1]
    return np.ascontiguousarray(
        mat_T.reshape(KT, P, n).transpose(1, 0, 2).reshape(P, KT * n)
    ).astype(dt)


def prepare_in_maps(layout_x, text_x, mask, Wqkv, bqkv, Wq, bq, Wkv, bkv,
                    Wffn, bffn):
    """Host-side sharding/layout prep: per-core input maps + scatter metadata."""
    layout_x = np.asarray(layout_x, dtype=np.float32)
    text_x = np.asarray(text_x, dtype=np.float32)
    mask = np.asarray(mask, dtype=np.float32)
    Wqkv = np.asarray(Wqkv, dtype=np.float64)
    bqkv = np.asarray(bqkv, dtype=np.float64).reshape(3 * C)
    Wq = np.asarray(Wq, dtype=np.float64)
    bq = np.asarray(bq, dtype=np.float64).reshape(C)
    Wkv = np.asarray(Wkv, dtype=np.float64)
    bkv = np.asarray(bkv, dtype=np.float64).reshape(2 * C)
    Wffn = np.asarray(Wffn, dtype=np.float64)
    bffn = np.asarray(bffn, dtype=np.float64).reshape(C)

    Wq1, Wk1, Wv1 = Wqkv[:, :C], Wqkv[:, C : 2 * C], Wqkv[:, 2 * C :]
    Wk2, Wv2 = Wkv[:, :C], Wkv[:, C : 2 * C]
    bv1, bv2 = bqkv[2 * C :], bkv[C:]
    # exact algebraic folds (host, fp64)
    W1f = SCALE * (Wq1 @ Wk1.T)  # z1 fold
    W2t = SCALE * (Wq @ Wk2.T)  # q2/k2 fold
    W2pp = Wv1 @ W2t  # v1 fold: attn1 emits z2 directly
    b2pp = bv1 @ W2t
    Wvf = Wv2 @ Wffn  # v2 fold: attn2 emits the final output
    bvf = bv2 @ Wffn + bffn
    g1 = Wk1 @ bqkv[:C]  # per-key score bias from bq1 (zero in practice)
    g2 = Wk2 @ bq

    w1 = _pack(np.clip(W1f * W1SCALE, -240, 240), _np_of(WT_DT))
    w2v_b = _pack(W2pp, _np_of(WN_DT))
    wvf_b = _pack(Wvf, _np_of(WN_DT))
    b2v = b2pp.reshape(1, C).astype(np.float32)
    bvf_r = bvf.reshape(1, C).astype(np.float32)

    B = layout_x.shape[0]
    counts = [int((mask[b] != 0).sum()) for b in range(B)]
    set_nv(max(max(counts), 1))
    in_maps, metas = [], []
    for b in range(B):
        idx = np.nonzero(mask[b])[0]
        nv = len(idx)
        assert 0 < nv <= NV, f"valid count {nv} outside (0, {NV}]"
        xc = np.zeros((NV, C), np.float32)
        xc[:nv] = layout_x[b][idx]
        tc_ = np.zeros((NV, C), np.float32)
        tc_[:nv] = text_x[b][idx]
        mc = np.zeros(MT * P, np.float32)
        mc[:nv] = 1.0
        colb1 = -10000.0 * (1.0 - mc)
        colb1[:NV] += SCALE * (xc @ g1)
        colb2 = -10000.0 * (1.0 - mc)
        colb2[:NV] += SCALE * (tc_ @ g2)
        mean_t = text_x[b].astype(np.float64).mean(axis=0)  # over ALL rows
        outm = ((mean_t @ Wv2 + bv2) @ Wffn + bffn).astype(np.float32)
        in_maps.append({
            "xT": _pack(xc.T, _np_of(AD)),
            "tT": _pack(tc_.T, _np_of(AD)),
            "w1": w1, "w2v": w2v_b, "b2v": b2v,
            "wvf": wvf_b, "bvf": bvf_r,
            "colb1": np.ascontiguousarray(
                colb1.astype(np.float32).reshape(MT, P).T),
            "colb2": np.ascontiguousarray(
                colb2.astype(np.float32).reshape(MT, P).T),
        })
        metas.append((idx, nv, outm))
    return in_maps, metas


_NC_CACHE = {}


def _get_nc():
    if NV not in _NC_CACHE:
        _NC_CACHE[NV] = build_nc()
    return _NC_CACHE[NV]


def kernel(layout_x, text_x, mask, Wqkv, bqkv, Wq, bq, Wkv, bkv, Wffn, bffn):
    in_maps, metas = prepare_in_maps(
        layout_x, text_x, mask, Wqkv, bqkv, Wq, bq, Wkv, bkv, Wffn, bffn
    )
    B = len(in_maps)
    assert B == N_CORES
    nc = _get_nc()
    res = run_bass_kernel_spmd(nc, in_maps, core_ids=list(range(N_CORES)))
    mask = np.asarray(mask, dtype=np.float32)
    out = np.zeros((B, M, C), np.float32)
    for b in range(B):
        idx, nv, outm = metas[b]
        oc = np.asarray(res.results[b]["out"]).astype(np.float32)
        out[b][idx] = oc[:nv]
        out[b][mask[b] == 0] = outm
    return out
